# revision 13
# baseline (speedup 1.0000x reference)
"""Trainium2 Bass kernel for the vq_codebook / ClusteringLayer problem.

Computes, for inputs [N=200000, D=128] and clusters [K=256, D=128]:
    dist2 = ||x||^2 + ||c||^2 - 2 x.c          (GEMM trick)
    q     = 1 / (1 + dist2)                    (ALPHA=1 -> power term is q**1)
    q     = q / sum_k q                        (row normalize)

Sharding: data-parallel over N across 8 NeuronCores; the [K, D] codebook is
replicated.

v2 layout: the host uploads x already transposed ([D, rows] bf16) so the
matmul stationary operand comes straight from DMA (no PE transpose, no
ACT copy-cast), plus per-row ||x||^2 ([128, 196] f32). Output is stored
partition-major ([128, 196*256] bf16) and untransposed on the host.

Per 128-row tile (196 tiles/core, processed in chunks of 8):
    PE   : psum = xt_tile.T @ (-2 C^T)  (bf16, stationary reloaded per tile)
    DVE  : fused custom op: q = recip_1nr(psum + (1+csq) + xsq),
           row-sums as accum_out  (single pass, reads PSUM directly)
    DVE  : rsums = 1/sums per chunk (exact reciprocal)
    ACT/POOL : qn = q * rsums (tensor_scalar / activation-scale, bf16)
    DMA  : 2KB-per-partition loads, 4KB-per-partition stores
"""

import sys

if "/opt/trn_rl_repo" not in sys.path:
    sys.path.insert(0, "/opt/trn_rl_repo")

import numpy as np

N_FULL = 200000
D = 128
K = 256
N_CORES = 8
TILE_P = 128
N_PAD = 200704  # = 8 * 25088 = 8 * 196 * 128
ROWS_PER_CORE = N_PAD // N_CORES  # 25088
TILES_PER_CORE = ROWS_PER_CORE // TILE_P  # 196
CHUNKS = [8] * 24 + [4]  # tiles per pipeline chunk (sum = 196)

_PROGRAM = None
_FUSED_OP = None


def _register_fused_op():
    """Custom DVE op: out = recip_1nr(in0 + in1 + s0); accum_out = sum(out).

    in0 = PSUM cross term (-2 x.c), in1 = replicated (1 + ||c||^2) row,
    s0 = per-partition ||x||^2, s1/imm2 = minimax seed pair for a
    bitwise-NOT exponent-flip reciprocal seed plus one Newton step
    (~1.7e-3 max rel err over the value range here).
    """
    global _FUSED_OP
    if _FUSED_OP is not None:
        return _FUSED_OP
    import numpy as np
    from operator import add as _add
    from concourse.dve_spec import Spec, Src0, Src1, C0, C1, C2, Zero, AluOp, Bin
    from concourse import dve_ops

    name = "RECIP1NR_BCS_ACC"
    _t = (Src0 + Src1) + C0
    _ny = Bin(AluOp.BITWISE_NOT, _t, _t)
    _z0 = _ny * C1
    _z1 = _z0 * (C2 - _t * _z0)

    def _ref(in0, in1, c0, c1, c2):
        t = (in0.astype(np.float32) + in1 + c0).astype(np.float32)
        ny = (~t.view(np.int32)).view(np.float32)
        z0 = ny * np.float32(c1)
        b = (z0 * (np.float32(c2) - t * z0)).astype(np.float32)
        return b, b.reshape(b.shape[0], -1).sum(axis=-1, keepdims=True)

    op = dve_ops.DveOp(
        name,
        Spec(body=_z1, accum=_add, accum_init=Zero, reference=_ref),
        subdim=False,
        uops_sha={},
    )
    dve_ops.OPS.append(op)
    dve_ops._SUB_OPCODE_FOR_NAME[name] = (
        dve_ops._CUSTOM_DVE_ROW_BASE + len(dve_ops.OPS) - 1)
    dve_ops.CUSTOM_DVE_SPECS[name] = op.spec

    # pin the uops sha (computed locally; equivalent of test_ops_golden)
    from concourse.dve_spec import lower, _has_src1
    from concourse.dve_uop import DveOpSpec

    for ver in ("v3",):
        s = DveOpSpec(name=name, opcode=dve_ops.get_dve_sub_opcode(name),
                      uops=lower(op.spec, ver=ver), rd1_en=_has_src1(op.spec))
        op.uops_sha[ver] = s.sha(ver)
    _FUSED_OP = op
    return op


RECIP_C1 = -0.23549792
RECIP_C2 = 2.0017324


def _build_program_v2():
    import concourse.bass as bass
    import concourse.tile as tile
    from concourse import mybir, bacc

    fused = _register_fused_op()

    f32 = mybir.dt.float32
    bf16 = mybir.dt.bfloat16

    nc = bacc.Bacc("TRN2", target_bir_lowering=False, debug=False,
                   num_devices=N_CORES)

    xt_d = nc.dram_tensor("xt", [D, ROWS_PER_CORE], bf16,
                          kind="ExternalInput").ap()
    xsq_d = nc.dram_tensor("xsq", [TILE_P, TILES_PER_CORE], f32,
                           kind="ExternalInput").ap()
    ct_d = nc.dram_tensor("ct", [D, K], bf16, kind="ExternalInput").ap()
    # (1 + ||c||^2) replicated across all 128 partitions
    csqr_d = nc.dram_tensor("csqr", [TILE_P, K], f32, kind="ExternalInput").ap()
    # output, partition-major: q[p, t*K + k] = q_row(t*128+p, k)
    q_d = nc.dram_tensor("q", [TILE_P, TILES_PER_CORE * K], bf16,
                         kind="ExternalOutput").ap()

    with tile.TileContext(nc) as tc:
        with (
            tc.tile_pool(name="consts", bufs=1) as cpool,
            tc.tile_pool(name="xin", bufs=4) as xin_pool,
            tc.tile_pool(name="qq", bufs=20) as q_pool,
            tc.tile_pool(name="qn", bufs=4) as qn_pool,
            tc.tile_pool(name="st", bufs=8) as st_pool,
            tc.tile_pool(name="psum_q", bufs=8, space="PSUM") as psq_pool,
        ):
            ct_s = cpool.tile([D, K], bf16)
            nc.sync.dma_start(ct_s[:], ct_d[:])
            csqr_s = cpool.tile([TILE_P, K], f32)
            nc.sync.dma_start(csqr_s[:], csqr_d[:])
            xsq_s = cpool.tile([TILE_P, TILES_PER_CORE], f32)
            nc.sync.dma_start(xsq_s[:], xsq_d[:])

            t0 = 0  # global tile index of chunk start
            for chunk in CHUNKS:
                xin_g = xin_pool.tile([TILE_P, chunk * TILE_P], bf16)
                nc.sync.dma_start(
                    xin_g[:], xt_d[:, t0 * TILE_P:(t0 + chunk) * TILE_P])

                qn_g = qn_pool.tile([TILE_P, chunk, K], bf16)
                sums_g = st_pool.tile([TILE_P, chunk], f32, tag="sumsg")
                rsums_g = st_pool.tile([TILE_P, chunk], f32, tag="rsumsg")
                q_tiles = []

                for t in range(chunk):
                    q_ps = psq_pool.tile([TILE_P, K], f32)
                    nc.tensor.matmul(q_ps[:],
                                     xin_g[:, t * TILE_P:(t + 1) * TILE_P],
                                     ct_s[:], start=True, stop=True)

                    # fused: q = recip_1nr(psum + csqr + xsq); sums = sum(q)
                    q_s = q_pool.tile([TILE_P, K], bf16)
                    nc.vector._custom_dve(
                        fused, out=q_s[:], in0=q_ps[:], in1=csqr_s[:],
                        s0=xsq_s[:, t0 + t:t0 + t + 1],
                        s1=RECIP_C1, imm2=RECIP_C2,
                        accum_out=sums_g[:, t:t + 1],
                    )
                    q_tiles.append(q_s)

                nc.vector.reciprocal(rsums_g[:], sums_g[:])

                for t in range(chunk):
                    if t == 0:
                        # DVE takes ~1/8 of muls (all-bf16: probes 2x rate)
                        nc.vector.tensor_scalar_mul(
                            qn_g[:, t, :], q_tiles[t][:],
                            rsums_g[:, t:t + 1])
                    else:
                        nc.scalar.activation(
                            qn_g[:, t, :], q_tiles[t][:],
                            mybir.ActivationFunctionType.Copy,
                            scale=rsums_g[:, t:t + 1])

                nc.sync.dma_start(
                    q_d[:, t0 * K:(t0 + chunk) * K], qn_g[:])
                t0 += chunk

    nc.compile()
    return nc


def _get_program():
    global _PROGRAM
    if _PROGRAM is None:
        _PROGRAM = _build_program_v2()
    return _PROGRAM


def kernel(inputs: np.ndarray, clusters: np.ndarray) -> np.ndarray:
    from concourse import bass_utils
    import ml_dtypes

    bf16 = ml_dtypes.bfloat16

    inputs = np.ascontiguousarray(inputs, dtype=np.float32)
    clusters = np.ascontiguousarray(clusters, dtype=np.float32)

    nc = _get_program()

    x_pad = np.zeros((N_PAD, D), dtype=np.float32)
    x_pad[:N_FULL] = inputs
    xt_full = x_pad.astype(bf16).T  # [D, N_PAD] view
    xsq_full = np.einsum("nd,nd->n", x_pad, x_pad).astype(np.float32)

    ct = np.ascontiguousarray((-2.0 * clusters.T).astype(bf16))
    csq1 = (1.0 + np.sum(clusters.astype(np.float64) ** 2, axis=1)).astype(
        np.float32)  # [K]
    csqr = np.ascontiguousarray(np.broadcast_to(csq1[None, :], (TILE_P, K)))

    in_maps = []
    for c in range(N_CORES):
        r0 = c * ROWS_PER_CORE
        xt = np.ascontiguousarray(xt_full[:, r0:r0 + ROWS_PER_CORE])
        # xsq[p, t] = ||x_{r0 + t*128 + p}||^2
        xsq = np.ascontiguousarray(
            xsq_full[r0:r0 + ROWS_PER_CORE].reshape(TILES_PER_CORE, TILE_P).T)
        in_maps.append({"xt": xt, "xsq": xsq, "ct": ct, "csqr": csqr})

    res = bass_utils.run_bass_kernel_spmd(nc, in_maps,
                                          core_ids=list(range(N_CORES)))
    # q_dev [128, 196*256] bf16, partition-major -> [rows, K] f32
    out = np.empty((N_PAD, K), dtype=np.float32)
    for c in range(N_CORES):
        q_dev = res.results[c]["q"].reshape(TILE_P, TILES_PER_CORE, K)
        out[c * ROWS_PER_CORE:(c + 1) * ROWS_PER_CORE] = (
            q_dev.transpose(1, 0, 2).reshape(ROWS_PER_CORE, K).astype(
                np.float32))
    return np.ascontiguousarray(out[:N_FULL])


# revision 16
# speedup vs baseline: 1.0184x; 1.0184x over previous
"""Trainium2 Bass kernel for the vq_codebook / ClusteringLayer problem.

Computes, for inputs [N=200000, D=128] and clusters [K=256, D=128]:
    dist2 = ||x||^2 + ||c||^2 - 2 x.c          (GEMM trick)
    q     = 1 / (1 + dist2)                    (ALPHA=1 -> power term is q**1)
    q     = q / sum_k q                        (row normalize)

Sharding: data-parallel over N across 8 NeuronCores; the [K, D] codebook is
replicated.

v2 layout: the host uploads x already transposed ([D, rows] bf16) so the
matmul stationary operand comes straight from DMA (no PE transpose, no
ACT copy-cast), plus per-row ||x||^2 ([128, 196] f32). Output is stored
partition-major ([128, 196*256] bf16) and untransposed on the host.

Per 128-row tile (196 tiles/core, processed in chunks of 8):
    PE   : psum = xt_tile.T @ (-2 C^T)  (bf16, stationary reloaded per tile)
    DVE  : fused custom op: q = recip_1nr(psum + (1+csq) + xsq),
           row-sums as accum_out  (single pass, reads PSUM directly)
    DVE  : rsums = 1/sums per chunk (exact reciprocal)
    ACT/POOL : qn = q * rsums (tensor_scalar / activation-scale, bf16)
    DMA  : 2KB-per-partition loads, 4KB-per-partition stores
"""

import sys

if "/opt/trn_rl_repo" not in sys.path:
    sys.path.insert(0, "/opt/trn_rl_repo")

import numpy as np

N_FULL = 200000
D = 128
K = 256
N_CORES = 8
TILE_P = 128
N_PAD = 200704  # = 8 * 25088 = 8 * 196 * 128
ROWS_PER_CORE = N_PAD // N_CORES  # 25088
TILES_PER_CORE = ROWS_PER_CORE // TILE_P  # 196
CHUNKS = [4] + [8] * 24  # tiles per pipeline chunk (sum = 196)

_PROGRAM = None
_FUSED_OP = None


def _register_fused_op():
    """Custom DVE op: out = recip_1nr(in0 + in1 + s0); accum_out = sum(out).

    in0 = PSUM cross term (-2 x.c), in1 = replicated (1 + ||c||^2) row,
    s0 = per-partition ||x||^2, s1/imm2 = minimax seed pair for a
    bitwise-NOT exponent-flip reciprocal seed plus one Newton step
    (~1.7e-3 max rel err over the value range here).
    """
    global _FUSED_OP
    if _FUSED_OP is not None:
        return _FUSED_OP
    import numpy as np
    from operator import add as _add
    from concourse.dve_spec import Spec, Src0, Src1, C0, C1, C2, Zero, AluOp, Bin
    from concourse import dve_ops

    name = "RECIP1NR_BCS_ACC"
    _t = (Src0 + Src1) + C0
    _ny = Bin(AluOp.BITWISE_NOT, _t, _t)
    _z0 = _ny * C1
    _z1 = _z0 * (C2 - _t * _z0)

    def _ref(in0, in1, c0, c1, c2):
        t = (in0.astype(np.float32) + in1 + c0).astype(np.float32)
        ny = (~t.view(np.int32)).view(np.float32)
        z0 = ny * np.float32(c1)
        b = (z0 * (np.float32(c2) - t * z0)).astype(np.float32)
        return b, b.reshape(b.shape[0], -1).sum(axis=-1, keepdims=True)

    op = dve_ops.DveOp(
        name,
        Spec(body=_z1, accum=_add, accum_init=Zero, reference=_ref),
        subdim=False,
        uops_sha={},
    )
    dve_ops.OPS.append(op)
    dve_ops._SUB_OPCODE_FOR_NAME[name] = (
        dve_ops._CUSTOM_DVE_ROW_BASE + len(dve_ops.OPS) - 1)
    dve_ops.CUSTOM_DVE_SPECS[name] = op.spec

    # pin the uops sha (computed locally; equivalent of test_ops_golden)
    from concourse.dve_spec import lower, _has_src1
    from concourse.dve_uop import DveOpSpec

    for ver in ("v3",):
        s = DveOpSpec(name=name, opcode=dve_ops.get_dve_sub_opcode(name),
                      uops=lower(op.spec, ver=ver), rd1_en=_has_src1(op.spec))
        op.uops_sha[ver] = s.sha(ver)
    _FUSED_OP = op
    return op


RECIP_C1 = -0.23549792
RECIP_C2 = 2.0017324


def _build_program_v2():
    import concourse.bass as bass
    import concourse.tile as tile
    from concourse import mybir, bacc

    fused = _register_fused_op()

    f32 = mybir.dt.float32
    bf16 = mybir.dt.bfloat16

    nc = bacc.Bacc("TRN2", target_bir_lowering=False, debug=False,
                   num_devices=N_CORES)

    xt_d = nc.dram_tensor("xt", [D, ROWS_PER_CORE], bf16,
                          kind="ExternalInput").ap()
    xsq_d = nc.dram_tensor("xsq", [TILE_P, TILES_PER_CORE], f32,
                           kind="ExternalInput").ap()
    ct_d = nc.dram_tensor("ct", [D, K], bf16, kind="ExternalInput").ap()
    # (1 + ||c||^2) replicated across all 128 partitions
    csqr_d = nc.dram_tensor("csqr", [TILE_P, K], f32, kind="ExternalInput").ap()
    # output, partition-major: q[p, t*K + k] = q_row(t*128+p, k)
    q_d = nc.dram_tensor("q", [TILE_P, TILES_PER_CORE * K], bf16,
                         kind="ExternalOutput").ap()

    with tile.TileContext(nc) as tc:
        with (
            tc.tile_pool(name="consts", bufs=1) as cpool,
            tc.tile_pool(name="xin", bufs=4) as xin_pool,
            tc.tile_pool(name="qq", bufs=20) as q_pool,
            tc.tile_pool(name="qn", bufs=4) as qn_pool,
            tc.tile_pool(name="st", bufs=8) as st_pool,
            tc.tile_pool(name="psum_q", bufs=8, space="PSUM") as psq_pool,
        ):
            # issue order matters: the first matmul needs ct + chunk-0 x;
            # csqr/xsq are only needed by the first DVE op ~2us later
            ct_s = cpool.tile([D, K], bf16)
            nc.sync.dma_start(ct_s[:], ct_d[:])
            xin0 = xin_pool.tile([TILE_P, CHUNKS[0] * TILE_P], bf16)
            nc.sync.dma_start(xin0[:], xt_d[:, :CHUNKS[0] * TILE_P])
            csqr_s = cpool.tile([TILE_P, K], f32)
            nc.sync.dma_start(csqr_s[:], csqr_d[:])
            xsq_s = cpool.tile([TILE_P, TILES_PER_CORE], f32)
            nc.sync.dma_start(xsq_s[:], xsq_d[:])

            t0 = 0  # global tile index of chunk start
            for ci, chunk in enumerate(CHUNKS):
                if ci == 0:
                    xin_g = xin0
                else:
                    xin_g = xin_pool.tile([TILE_P, chunk * TILE_P], bf16)
                    nc.sync.dma_start(
                        xin_g[:], xt_d[:, t0 * TILE_P:(t0 + chunk) * TILE_P])

                qn_g = qn_pool.tile([TILE_P, chunk, K], bf16)
                sums_g = st_pool.tile([TILE_P, chunk], f32, tag="sumsg")
                rsums_g = st_pool.tile([TILE_P, chunk], f32, tag="rsumsg")
                q_tiles = []

                for t in range(chunk):
                    q_ps = psq_pool.tile([TILE_P, K], f32)
                    nc.tensor.matmul(q_ps[:],
                                     xin_g[:, t * TILE_P:(t + 1) * TILE_P],
                                     ct_s[:], start=True, stop=True)

                    # fused: q = recip_1nr(psum + csqr + xsq); sums = sum(q)
                    q_s = q_pool.tile([TILE_P, K], bf16)
                    nc.vector._custom_dve(
                        fused, out=q_s[:], in0=q_ps[:], in1=csqr_s[:],
                        s0=xsq_s[:, t0 + t:t0 + t + 1],
                        s1=RECIP_C1, imm2=RECIP_C2,
                        accum_out=sums_g[:, t:t + 1],
                    )
                    q_tiles.append(q_s)

                nc.vector.reciprocal(rsums_g[:], sums_g[:])

                # DVE takes ~1/8 of muls (engine balance); the final chunk
                # shifts more muls to DVE to shorten the ACT drain tail
                n_dve = 3 if ci == len(CHUNKS) - 1 else 1
                for t in range(chunk):
                    if t < n_dve:
                        nc.vector.tensor_scalar_mul(
                            qn_g[:, t, :], q_tiles[t][:],
                            rsums_g[:, t:t + 1])
                    else:
                        nc.scalar.activation(
                            qn_g[:, t, :], q_tiles[t][:],
                            mybir.ActivationFunctionType.Copy,
                            scale=rsums_g[:, t:t + 1])

                nc.sync.dma_start(
                    q_d[:, t0 * K:(t0 + chunk) * K], qn_g[:])
                t0 += chunk

    nc.compile()
    return nc


def _get_program():
    global _PROGRAM
    if _PROGRAM is None:
        _PROGRAM = _build_program_v2()
    return _PROGRAM


def kernel(inputs: np.ndarray, clusters: np.ndarray) -> np.ndarray:
    from concourse import bass_utils
    import ml_dtypes

    bf16 = ml_dtypes.bfloat16

    inputs = np.ascontiguousarray(inputs, dtype=np.float32)
    clusters = np.ascontiguousarray(clusters, dtype=np.float32)

    nc = _get_program()

    x_pad = np.zeros((N_PAD, D), dtype=np.float32)
    x_pad[:N_FULL] = inputs
    xt_full = x_pad.astype(bf16).T  # [D, N_PAD] view
    xsq_full = np.einsum("nd,nd->n", x_pad, x_pad).astype(np.float32)

    ct = np.ascontiguousarray((-2.0 * clusters.T).astype(bf16))
    csq1 = (1.0 + np.sum(clusters.astype(np.float64) ** 2, axis=1)).astype(
        np.float32)  # [K]
    csqr = np.ascontiguousarray(np.broadcast_to(csq1[None, :], (TILE_P, K)))

    in_maps = []
    for c in range(N_CORES):
        r0 = c * ROWS_PER_CORE
        xt = np.ascontiguousarray(xt_full[:, r0:r0 + ROWS_PER_CORE])
        # xsq[p, t] = ||x_{r0 + t*128 + p}||^2
        xsq = np.ascontiguousarray(
            xsq_full[r0:r0 + ROWS_PER_CORE].reshape(TILES_PER_CORE, TILE_P).T)
        in_maps.append({"xt": xt, "xsq": xsq, "ct": ct, "csqr": csqr})

    res = bass_utils.run_bass_kernel_spmd(nc, in_maps,
                                          core_ids=list(range(N_CORES)))
    # q_dev [128, 196*256] bf16, partition-major -> [rows, K] f32
    out = np.empty((N_PAD, K), dtype=np.float32)
    for c in range(N_CORES):
        q_dev = res.results[c]["q"].reshape(TILE_P, TILES_PER_CORE, K)
        out[c * ROWS_PER_CORE:(c + 1) * ROWS_PER_CORE] = (
            q_dev.transpose(1, 0, 2).reshape(ROWS_PER_CORE, K).astype(
                np.float32))
    return np.ascontiguousarray(out[:N_FULL])
